# revision 9
# baseline (speedup 1.0000x reference)
"""Cross-attention layer on 8 Trainium2 NeuronCores via Bass/Tile.

Problem: q/k/v = Linear(zt/ic/ic); softmax(q k^T / sqrt(64)) v;  B=4, L=2048,
D=1024, H=16 heads of 64.

Sharding: core c -> batch b = c//2, head-group g = c%2 (8 heads, d-slice of
512). Host pre-transposes activations/weights so every matmul contracts over
the partition dim, and appends a ones column per head to V so the softmax
denominator rides along the attn@v matmul (row 64 of the [65, 512] psum).

All matmuls use float32r (full-rate fp32 storage, ~1e-3 matmul rounding).
Scores are computed transposed ([k, q] layout) so exp'd tiles feed attn@v
directly as the stationary operand with no on-chip transpose of the 4M-element
attention matrix; only the final [65, 512] outT tiles are PE-transposed back
to natural [q, d] layout. Softmax max-subtraction is skipped: scores ~N(0,1),
max < 7, exp stays comfortably in fp32 range.
"""
import sys
import types

import numpy as np

B, LQ, LK, D, H = 4, 2048, 2048, 1024, 16
HD = 64
NCORES = 8
GD = 512          # d-dims per core group (8 heads)
SCALE = 0.125     # 1/sqrt(64), exact power of two -> folded into Wq/bq

_cached = {}


def _build():
    import concourse.bass as bass  # noqa: F401
    import concourse.tile as tile
    from concourse import bacc, mybir

    f32 = mybir.dt.float32
    f32r = mybir.dt.float32r
    EXP = mybir.ActivationFunctionType.Exp

    nc = bacc.Bacc("TRN2", target_bir_lowering=False, debug=False,
                   num_devices=NCORES)
    ztT = nc.dram_tensor("ztT", [D, LQ], f32r, kind="ExternalInput").ap()
    icT = nc.dram_tensor("icT", [D, LK], f32r, kind="ExternalInput").ap()
    wq = nc.dram_tensor("wq", [D, GD], f32r, kind="ExternalInput").ap()
    wk = nc.dram_tensor("wk", [D, GD], f32r, kind="ExternalInput").ap()
    wv = nc.dram_tensor("wv", [D, 520], f32r, kind="ExternalInput").ap()
    wqb = nc.dram_tensor("wqb", [1, GD], f32r, kind="ExternalInput").ap()
    wkb = nc.dram_tensor("wkb", [1, GD], f32r, kind="ExternalInput").ap()
    wvb = nc.dram_tensor("wvb", [1, 520], f32r, kind="ExternalInput").ap()
    o = nc.dram_tensor("o", [8, LQ, HD], f32, kind="ExternalOutput").ap()

    from contextlib import ExitStack
    with tile.TileContext(nc) as tc, ExitStack() as stk:
        singles = stk.enter_context(tc.tile_pool(name="singles", bufs=1))
        ones_f = singles.tile([1, 512], f32)
        nc.vector.memset(ones_f, 1.0)
        ones_row = singles.tile([1, 512], f32r)
        nc.vector.tensor_copy(ones_row, ones_f)
        from concourse.masks import make_identity
        ident = singles.tile([128, 128], f32)
        make_identity(nc, ident)

        wqb_sb = singles.tile([1, GD], f32r)
        wkb_sb = singles.tile([1, GD], f32r)
        wvb_sb = singles.tile([1, 520], f32r)
        nc.sync.dma_start(out=wqb_sb, in_=wqb)
        nc.sync.dma_start(out=wkb_sb, in_=wkb)
        nc.sync.dma_start(out=wvb_sb, in_=wvb)

        persist = stk.enter_context(tc.tile_pool(name="persist", bufs=1))
        qT_sb = [persist.tile([128, LQ], f32r, name=f"qT{t}") for t in range(4)]
        kT_sb = [persist.tile([128, LK], f32r, name=f"kT{t}") for t in range(4)]
        v_sb = [persist.tile([128, 520], f32r, name=f"v{i}") for i in range(16)]

        # ---- phase 1: qT = (wq^T zt^T scaled) [d, lq], 4 d-tiles ----
        with tc.tile_pool(name="ztp", bufs=1) as ztp, \
             tc.tile_pool(name="wqp", bufs=1) as wqp, \
             tc.tile_pool(name="pj", bufs=4, space="PSUM") as pj:
            zt_t = [ztp.tile([128, LQ], f32r, name=f"zt{e}") for e in range(8)]
            wq_t = [wqp.tile([128, GD], f32r, name=f"wqt{e}") for e in range(8)]
            for e in range(8):
                nc.sync.dma_start(out=zt_t[e], in_=ztT[e*128:(e+1)*128, :])
                nc.sync.dma_start(out=wq_t[e], in_=wq[e*128:(e+1)*128, :])
            for t in range(4):
                for lc in range(4):
                    pp = pj.tile([128, 512], f32, tag="pj")
                    for e in range(8):
                        nc.tensor.matmul(pp, wq_t[e][:, t*128:(t+1)*128],
                                         zt_t[e][:, lc*512:(lc+1)*512],
                                         start=(e == 0), stop=False)
                    nc.tensor.matmul(pp, wqb_sb[0:1, t*128:(t+1)*128],
                                     ones_row, start=False, stop=True)
                    nc.vector.tensor_copy(qT_sb[t][:, lc*512:(lc+1)*512], pp)

        # ---- phase 2: kT + v from icT (two lk-halves to bound SBUF) ----
        with tc.tile_pool(name="icp", bufs=1) as icp, \
             tc.tile_pool(name="wkp", bufs=1) as wkp, \
             tc.tile_pool(name="pj2", bufs=2, space="PSUM") as pj2:
            ic_t = [icp.tile([128, 1024], f32r, name=f"ic{e}") for e in range(8)]
            wk_t = [wkp.tile([128, GD], f32r, name=f"wkt{e}") for e in range(8)]
            wv_t = [wkp.tile([128, 520], f32r, name=f"wvt{e}") for e in range(8)]
            for e in range(8):
                nc.sync.dma_start(out=wk_t[e], in_=wk[e*128:(e+1)*128, :])
                nc.sync.dma_start(out=wv_t[e], in_=wv[e*128:(e+1)*128, :])
            for half in range(2):
                l0 = half * 1024
                for e in range(8):
                    nc.sync.dma_start(out=ic_t[e],
                                      in_=icT[e*128:(e+1)*128, l0:l0+1024])
                for t in range(4):
                    for lc in range(2):
                        pp = pj2.tile([128, 512], f32, tag="pj2")
                        for e in range(8):
                            nc.tensor.matmul(pp, wk_t[e][:, t*128:(t+1)*128],
                                             ic_t[e][:, lc*512:(lc+1)*512],
                                             start=(e == 0), stop=False)
                        nc.tensor.matmul(pp, wkb_sb[0:1, t*128:(t+1)*128],
                                         ones_row, start=False, stop=True)
                        nc.vector.tensor_copy(
                            kT_sb[t][:, l0+lc*512:l0+(lc+1)*512], pp)
                for kt in range(8):
                    vp = pj2.tile([128, 520], f32, tag="vpj")
                    for e in range(8):
                        nc.tensor.matmul(vp[:, 0:512],
                                         ic_t[e][:, kt*128:(kt+1)*128],
                                         wv_t[e][:, 0:512],
                                         start=(e == 0), stop=False)
                        nc.tensor.matmul(vp[:, 512:520],
                                         ic_t[e][:, kt*128:(kt+1)*128],
                                         wv_t[e][:, 512:520],
                                         start=(e == 0), stop=False)
                    nc.tensor.matmul(vp[:, 0:512], ones_row[0:1, 0:128],
                                     wvb_sb[0:1, 0:512], start=False, stop=True)
                    nc.tensor.matmul(vp[:, 512:520], ones_row[0:1, 0:128],
                                     wvb_sb[0:1, 512:520], start=False,
                                     stop=True)
                    nc.vector.tensor_copy(v_sb[half*8+kt], vp)

        # ---- phase 3: attention ----
        GROUPS = [(0, 4), (4, 2), (6, 4), (10, 2), (12, 4)]
        with tc.tile_pool(name="sca", bufs=1, space="PSUM") as sca, \
             tc.tile_pool(name="scb", bufs=1, space="PSUM") as scb, \
             tc.tile_pool(name="otp", bufs=1, space="PSUM") as otp, \
             tc.tile_pool(name="trp", bufs=1, space="PSUM") as trp, \
             tc.tile_pool(name="exp", bufs=3) as expp, \
             tc.tile_pool(name="oap", bufs=2) as oap, \
             tc.tile_pool(name="recp", bufs=4) as recp, \
             tc.tile_pool(name="stg", bufs=2) as stgp:
            for t in range(4):
                for hh in range(2):
                    h = 2*t + hh
                    p0 = 64 * hh
                    stage = stgp.tile([128, 16, HD], f32, tag="stage")
                    for qc in range(4):
                        q0 = qc * 512
                        ot = otp.tile([65, 512], f32, tag="ot")
                        for gi, (k0, glen) in enumerate(GROUPS):
                            pool = sca if gi % 2 == 0 else scb
                            tag = "sa" if gi % 2 == 0 else "sb"
                            sc = pool.tile([128, glen*512], f32, tag=tag)
                            for j in range(glen):
                                kt = k0 + j
                                nc.tensor.matmul(
                                    sc[:, j*512:(j+1)*512],
                                    kT_sb[t][p0:p0+64, kt*128:(kt+1)*128],
                                    qT_sb[t][p0:p0+64, q0:q0+512],
                                    start=True, stop=True)
                            ex = expp.tile([128, glen*512], f32r, tag="ex")
                            nc.scalar.activation(out=ex, in_=sc, func=EXP)
                            for j in range(glen):
                                kt = k0 + j
                                nc.tensor.matmul(
                                    ot, v_sb[kt][:, h*65:(h+1)*65],
                                    ex[:, j*512:(j+1)*512],
                                    start=(kt == 0), stop=(kt == 15),
                                    skip_group_check=True)
                        oa = oap.tile([65, 512], f32, tag="oa")
                        nc.vector.tensor_copy(oa, ot)
                        for blk in range(4):
                            tr = trp.tile([128, 65], f32, tag="tr")
                            nc.tensor.transpose(tr, oa[:, blk*128:(blk+1)*128],
                                                ident[0:65, 0:65])
                            rec = recp.tile([128, 1], f32, tag="rec")
                            nc.vector.reciprocal(rec, tr[:, 64:65])
                            nc.vector.tensor_scalar_mul(
                                stage[:, qc*4+blk, :], tr[:, 0:64], rec)
                    nc.sync.dma_start(
                        out=o[h].rearrange("(t p) d -> p t d", p=128),
                        in_=stage)
    nc.finalize()
    return nc


def _prep_inputs(zt, ic, Wq, bq, Wk, bk, Wv, bv):
    """Build per-core input maps (host-side sharding + layout prep)."""
    zt = np.asarray(zt, dtype=np.float32)
    ic = np.asarray(ic, dtype=np.float32)
    in_maps = []
    for c in range(NCORES):
        b, g = c // 2, c % 2
        gs = slice(g*GD, (g+1)*GD)
        wv_aug = np.zeros((D, 520), np.float32)
        wvb_aug = np.zeros((1, 520), np.float32)
        Wvg = np.asarray(Wv[gs], np.float32)
        bvg = np.asarray(bv[gs], np.float32)
        for h in range(8):
            wv_aug[:, h*65:h*65+64] = Wvg[h*64:(h+1)*64, :].T
            wvb_aug[0, h*65:h*65+64] = bvg[h*64:(h+1)*64]
            wvb_aug[0, h*65+64] = 1.0
        in_maps.append({
            "ztT": np.ascontiguousarray(zt[b].T),
            "icT": np.ascontiguousarray(ic[b].T),
            "wq": np.ascontiguousarray((np.asarray(Wq[gs], np.float32)
                                        * SCALE).T),
            "wk": np.ascontiguousarray(np.asarray(Wk[gs], np.float32).T),
            "wv": wv_aug,
            "wqb": (np.asarray(bq[gs], np.float32) * SCALE)[None, :],
            "wkb": np.asarray(bk[gs], np.float32)[None, :],
            "wvb": wvb_aug,
        })
    return in_maps


def _run(in_maps, trace=False, tmpdir=None):
    if 'antenv.axon_hooks' not in sys.modules:
        try:
            from trn_agent_boot.trn_boot import _ntff_profile_via_ctypes
            mod = types.ModuleType('antenv.axon_hooks')
            hook = _ntff_profile_via_ctypes('/opt/axon/libaxon_pjrt.so')
            mod.get_axon_ntff_profile_hook = lambda: hook
            mod.set_axon_ntff_profile_hook = lambda h: None
            sys.modules['antenv.axon_hooks'] = mod
        except Exception:
            pass
    from concourse import bass_utils
    bass_utils.upload_artifacts = lambda d: "local://skipped"
    if 'nc' not in _cached:
        _cached['nc'] = _build()
    return bass_utils.run_bass_kernel_spmd(
        _cached['nc'], in_maps, core_ids=list(range(NCORES)),
        trace=trace, tmpdir=tmpdir)


def kernel(zt, ic, Wq, bq, Wk, bk, Wv, bv, _trace=False, _tmpdir=None):
    in_maps = _prep_inputs(zt, ic, Wq, bq, Wk, bk, Wv, bv)
    res = _run(in_maps, trace=_trace, tmpdir=_tmpdir)
    out = np.empty((B, LQ, D), np.float32)
    for c in range(NCORES):
        b, g = c // 2, c % 2
        oc = res.results[c]["o"]          # [8, LQ, 64]
        out[b, :, g*GD:(g+1)*GD] = oc.transpose(1, 0, 2).reshape(LQ, GD)
    kernel.last_result = res
    return out
